# revision 19
# baseline (speedup 1.0000x reference)
"""Conv-RFF Trainium2 kernel: grouped 3x3/s2 conv (10 MC groups sharing input)
+ cos/sin random-feature epilogue, data-parallel over batch on 8 NeuronCores.

The end-to-end time is dominated by the axon tunnel (~20-35 MB/s, ~0.1-0.2 s
per RPC), so the kernel is organized around minimizing and pipelining
host<->device traffic:
  - x is quantized to int8 (2.1 MB up; the 1/XS dequant scale is folded into
    the fp16 weights, which the PE array consumes after an on-device upcast).
  - Weights upload 1/8th per core (92 KB total) and are reassembled on
    device by a small AllGather.
  - The conv runs as 2 accumulating fp16 matmuls per 512-pixel block into
    fp32 PSUM (row-gathered Kb layout), the epilogue does the
    round-to-nearest trig trick on ScalarE Sin, and cos/sin are quantized
    to int8 via the fp32 magic constant (exact round-to-nearest).
  - An in-kernel AllGather concatenates the 8 per-core results; the output
    is split into N_OUTS tensors so the host fetches slice k from core k
    in parallel threads (the tunnel caps at an aggregate bandwidth, but
    parallel streams overlap the per-RPC latency).
  - The whole thing runs TWICE per call (MC groups 0-4 and 5-9, same NEFF,
    different weight args): run 2's upload+exec RPC is dispatched async and
    overlaps run 1's output fetch, hiding the exec latency.
  - The jitted executable, the NEFF, and the dummy output-operand buffers
    (never donated, so reusable) are cached module-globally; steady-state
    calls do no recompilation.

Conv layout per core (one batch image each):
  - Kb[96, 64, 128] fp16 in SBUF via 6 DMAs:
      partitions 0..47  = "G"  = x rows (2*oy+ki-1) unshifted
      partitions 48..95 = "Gs" = x rows (2*oy+ki-1) shifted right 1 col
  - mm1: K=96  lhsT=W[kj1|kj0] rhs=Kb[:, oy, 0:128:2]   (even cols)
    mm2: K=48  lhsT=W[kj2]     rhs=Kb[0:48, oy, 1:128:2] (odd cols)
  - Weights pre-scaled by 1/(2*pi*XS) so PSUM holds u = ph/(2*pi) ("turns");
    round via the fp32 magic constant, then ScalarE Sin on the reduced
    argument (|arg| <= pi where HW Sin is exact), quarter-turn shift for cos.
"""

import sys

sys.path.insert(0, "/opt/trn_rl_repo")

import numpy as np

MC, IN_C, OUT_C, KK = 10, 16, 32, 3
D = IN_C * KK * KK  # 144
B, H, W = 8, 128, 128
HO = 64  # output spatial
N_RF = OUT_C * HO * HO
PI = float(np.pi)
MAGIC = float(1.5 * 2**23)
N_CORES = 8
MCP = 5  # MC groups per run; the program runs MC/MCP times per call
N_OUTS = 2  # output split per run, fetched from distinct cores in parallel
QS = 127.0  # int8 quantization scale for cos/sin values
XS = 127.0 / 4.0  # int8 quantization scale for x (clips at 4 sigma)
WCOL = MCP * OUT_C  # weight columns per run (160)


def _build_program(JW: int = 2, SB_BUFS: int = 3, PS_BUFS: int = 3):
    import concourse.bacc as bacc
    import concourse.mybir as mybir
    from concourse.tile import TileContext

    f32 = mybir.dt.float32
    f16 = mybir.dt.float16
    i8 = mybir.dt.int8
    AF = mybir.ActivationFunctionType
    ALU = mybir.AluOpType

    nc = bacc.Bacc("TRN2", target_bir_lowering=False, num_devices=N_CORES)
    xb = nc.dram_tensor("xb", [IN_C, H, W], i8, kind="ExternalInput")
    # each core uploads 1/8th of the weights; an AllGather reassembles them
    wp = nc.dram_tensor("wp", [96 // N_CORES, WCOL], f16, kind="ExternalInput")
    w2 = nc.dram_tensor("w2", [48 // N_CORES, WCOL], f16, kind="ExternalInput")
    bpo = N_CORES // N_OUTS  # batches per output tensor
    outs = [
        nc.dram_tensor(
            f"out{k}", [bpo, MCP, 2, OUT_C, 8, 512], i8, kind="ExternalOutput"
        )
        for k in range(N_OUTS)
    ]

    with TileContext(nc) as tc:
        with (
            tc.tile_pool(name="kbp", bufs=1) as kbp,
            tc.tile_pool(name="wpl", bufs=1) as wpl,
            tc.tile_pool(name="cst", bufs=1) as cst,
            tc.tile_pool(name="sb", bufs=SB_BUFS) as sb,
            tc.tile_pool(name="ob", bufs=4) as ob,
            tc.tile_pool(name="pp", bufs=PS_BUFS, space="PSUM") as pp,
            tc.tile_pool(name="dr", bufs=1, space="DRAM") as dr,
        ):
            loc = dr.tile([MCP, 2, OUT_C, 8, 512], i8)
            gath = dr.tile([N_CORES, MCP, 2, OUT_C, 8, 512], i8)
            kbs = kbp.tile([96, 64, 128], i8)
            wpt = wpl.tile([96, WCOL], f16)
            w2t = wpl.tile([48, WCOL], f16)
            hpi = cst.tile([128, 1], f32)
            nc.gpsimd.memset(hpi[:], PI / 2)
            # gather the per-core weight slices (collectives can't touch I/O
            # tensors, so bounce through internal DRAM on both sides)
            wpb = dr.tile([96 // N_CORES, WCOL], f16)
            w2b = dr.tile([48 // N_CORES, WCOL], f16)
            wpg = dr.tile([96, WCOL], f16)
            w2g = dr.tile([48, WCOL], f16)
            nc.sync.dma_start(wpb[:], wp[:])
            nc.sync.dma_start(w2b[:], w2[:])
            nc.gpsimd.collective_compute(
                "AllGather",
                mybir.AluOpType.bypass,
                replica_groups=[list(range(N_CORES))],
                ins=[wpb.opt()],
                outs=[wpg.opt()],
            )
            nc.gpsimd.collective_compute(
                "AllGather",
                mybir.AluOpType.bypass,
                replica_groups=[list(range(N_CORES))],
                ins=[w2b.opt()],
                outs=[w2g.opt()],
            )
            nc.sync.dma_start(wpt[:], wpg[:])
            nc.sync.dma_start(w2t[:], w2g[:])

            # zero padding slivers: shifted col 0, and oy=0 row for ki=0 taps
            nc.gpsimd.memset(kbs[32:64, :, 0:1], 0.0)
            nc.gpsimd.memset(kbs[64:96, :, 0:1], 0.0)
            nc.gpsimd.memset(kbs[0:16, 0:1, :], 0.0)
            nc.gpsimd.memset(kbs[32:64, 0:1, :], 0.0)

            # G (unshifted) at partitions 0..47; row sets per ki
            nc.sync.dma_start(kbs[0:16, 1:64, :], xb[:, 1:127:2, :])    # ki=0
            nc.sync.dma_start(kbs[16:32, :, :], xb[:, 0:128:2, :])      # ki=1
            nc.sync.dma_start(kbs[32:48, :, :], xb[:, 1:128:2, :])      # ki=2
            # Gs (shifted right 1 col) at partitions 48..95
            nc.sync.dma_start(kbs[48:64, 1:64, 1:128], xb[:, 1:127:2, 0:127])
            nc.sync.dma_start(kbs[64:80, :, 1:128], xb[:, 0:128:2, 0:127])
            nc.sync.dma_start(kbs[80:96, :, 1:128], xb[:, 1:128:2, 0:127])
            # upcast staged int8 image rows to fp16 for the PE array;
            # the 1/XS dequant scale is folded into the weights host-side
            kb = kbp.tile([96, 64, 128], f16)
            nc.scalar.copy(kb[:], kbs[:])

            chunks = [(0, 128), (128, WCOL - 128)]
            for c0, mcs in chunks:
                nmc = mcs // OUT_C
                mc0 = c0 // OUT_C
                for jb in range(8 // JW):
                    ps = pp.tile([mcs, JW, 512], f32, tag="ps")
                    for jj in range(JW):
                        j = jb * JW + jj
                        nc.tensor.matmul(
                            ps[:, jj, :],
                            wpt[:, c0 : c0 + mcs],
                            kb[:, j * 8 : (j + 1) * 8, 0:128:2],
                            start=True,
                            stop=False,
                        )
                        nc.tensor.matmul(
                            ps[:, jj, :],
                            w2t[:, c0 : c0 + mcs],
                            kb[0:48, j * 8 : (j + 1) * 8, 1:128:2],
                            start=False,
                            stop=True,
                        )
                    uc = sb.tile([mcs, JW, 512], f32, tag="uc")
                    nc.scalar.activation(uc[:], ps[:], AF.Identity)
                    t1s = sb.tile([mcs, JW, 512], f32, tag="t1s")
                    nc.gpsimd.tensor_scalar(t1s[:], uc[:], MAGIC, None, op0=ALU.add)
                    t1c = sb.tile([mcs, JW, 512], f32, tag="t1c")
                    nc.gpsimd.tensor_scalar(
                        t1c[:], uc[:], 0.25, MAGIC, op0=ALU.add, op1=ALU.add
                    )
                    gs = sb.tile([mcs, JW, 512], f32, tag="gs")
                    nc.vector.scalar_tensor_tensor(
                        gs[:], t1s[:], MAGIC, uc[:], op0=ALU.subtract, op1=ALU.subtract
                    )
                    gc = sb.tile([mcs, JW, 512], f32, tag="gc")
                    nc.vector.scalar_tensor_tensor(
                        gc[:], t1c[:], MAGIC, uc[:], op0=ALU.subtract, op1=ALU.subtract
                    )
                    sn = sb.tile([mcs, JW, 512], f32, tag="sn")
                    nc.scalar.activation(sn[:], gs[:], AF.Sin, scale=-2 * PI)
                    cs = sb.tile([mcs, JW, 512], f32, tag="cs")
                    nc.scalar.activation(
                        cs[:], gc[:], AF.Sin, bias=hpi[:mcs, :], scale=-2 * PI
                    )
                    # quantize to int8: q = round(127*v) via the magic constant
                    # (value ends integer-valued in f32, so the int8 convert is
                    # exact in any rounding mode)
                    qs_t = sb.tile([mcs, JW, 512], f32, tag="qs")
                    nc.gpsimd.tensor_scalar(
                        qs_t[:], sn[:], QS, MAGIC, op0=ALU.mult, op1=ALU.add
                    )
                    qc_t = sb.tile([mcs, JW, 512], f32, tag="qc")
                    nc.gpsimd.tensor_scalar(
                        qc_t[:], cs[:], QS, MAGIC, op0=ALU.mult, op1=ALU.add
                    )
                    sn8 = ob.tile([mcs, JW, 512], i8, tag="sn8")
                    nc.vector.tensor_scalar(
                        sn8[:], qs_t[:], MAGIC, None, op0=ALU.subtract
                    )
                    cs8 = ob.tile([mcs, JW, 512], i8, tag="cs8")
                    nc.vector.tensor_scalar(
                        cs8[:], qc_t[:], MAGIC, None, op0=ALU.subtract
                    )
                    for m in range(nmc):
                        nc.sync.dma_start(
                            loc[mc0 + m, 0, :, jb * JW : (jb + 1) * JW, :],
                            cs8[m * 32 : (m + 1) * 32, :, :],
                        )
                        nc.sync.dma_start(
                            loc[mc0 + m, 1, :, jb * JW : (jb + 1) * JW, :],
                            sn8[m * 32 : (m + 1) * 32, :, :],
                        )
            nc.gpsimd.collective_compute(
                "AllGather",
                mybir.AluOpType.bypass,
                replica_groups=[list(range(N_CORES))],
                ins=[loc.opt()],
                outs=[gath.opt()],
            )
            for k in range(N_OUTS):
                nc.sync.dma_start(
                    outs[k][:], gath[k * bpo : (k + 1) * bpo]
                )
    nc.compile()
    return nc


def _prep_weights(theta_logsigma, Omega_mean, Omega_logsigma, Omega_eps):
    om = Omega_eps.astype(np.float64) * np.exp(
        Omega_logsigma.astype(np.float64) * 0.5
    ) + Omega_mean.astype(np.float64)
    wd = om.transpose(1, 0, 2).reshape(D, MC * OUT_C)  # [d, mc*32+oc]
    wt = (wd / (2 * np.pi * XS)).reshape(KK, KK, IN_C, MC * OUT_C)
    kj0 = wt[:, 0].reshape(48, MC * OUT_C)
    kj1 = wt[:, 1].reshape(48, MC * OUT_C)
    kj2 = wt[:, 2].reshape(48, MC * OUT_C)
    wpair = np.ascontiguousarray(
        np.concatenate([kj1, kj0], axis=0), dtype=np.float16
    )
    wk2 = np.ascontiguousarray(kj2, dtype=np.float16)
    c_scale = float(np.exp(0.5 * float(theta_logsigma[0])) / np.sqrt(N_RF))
    return wpair, wk2, c_scale


_STATE = None


def _get_state():
    global _STATE
    if _STATE is not None:
        return _STATE

    import jax
    import jax.numpy as jnp
    from jax.sharding import Mesh, NamedSharding, PartitionSpec
    from jax.experimental.shard_map import shard_map
    from concourse import mybir
    from concourse import bass2jax
    from concourse.bass2jax import _bass_exec_p, install_neuronx_cc_hook

    nc = _build_program()
    install_neuronx_cc_hook()

    partition_name = nc.partition_id_tensor.name if nc.partition_id_tensor else None
    in_names, out_names, out_avals = [], [], []
    for alloc in nc.m.functions[0].allocations:
        if not isinstance(alloc, mybir.MemoryLocationSet):
            continue
        name = alloc.memorylocations[0].name
        if alloc.kind == "ExternalInput":
            if name != partition_name:
                in_names.append(name)
        elif alloc.kind == "ExternalOutput":
            out_names.append(name)
            shape = tuple(alloc.tensor_shape)
            dtype = mybir.dt.np(alloc.dtype)
            out_avals.append(jax.core.ShapedArray(shape, dtype))
    n_params = len(in_names)
    in_names_full = in_names + out_names
    if partition_name is not None:
        in_names_full.append(partition_name)

    def _body(*args):
        operands = list(args)
        if partition_name is not None:
            operands.append(bass2jax.partition_id_tensor())
        outs = _bass_exec_p.bind(
            *operands,
            out_avals=tuple(out_avals),
            in_names=tuple(in_names_full),
            out_names=tuple(out_names),
            lowering_input_output_aliases=(),
            sim_require_finite=True,
            sim_require_nnan=True,
            nc=nc,
        )
        return tuple(outs)

    devices = jax.devices()[:N_CORES]
    mesh = Mesh(np.asarray(devices), ("core",))
    n_outs = len(out_avals)
    in_specs = (PartitionSpec("core"),) * (n_params + n_outs)
    out_specs = (PartitionSpec("core"),) * n_outs
    # No donation: output operands are dummies (the NEFF writes every
    # element of every out tensor), so the same device-resident zero
    # buffers are reused every call with no re-upload.
    sharded = jax.jit(
        shard_map(
            _body, mesh=mesh, in_specs=in_specs, out_specs=out_specs, check_rep=False
        ),
        keep_unused=True,
    )

    zsharding = NamedSharding(mesh, PartitionSpec("core"))
    mkzeros = jax.jit(
        lambda: tuple(
            jnp.zeros((N_CORES * a.shape[0], *a.shape[1:]), a.dtype)
            for a in out_avals
        ),
        out_shardings=(zsharding,) * n_outs,
    )
    dummy_outs = jax.block_until_ready(mkzeros())

    _STATE = {
        "sharded": sharded,
        "dummy_outs": dummy_outs,
        "in_names": in_names,
        "out_names": out_names,
    }
    return _STATE


def kernel(x, theta_logsigma, Omega_mean, Omega_logsigma, Omega_eps):
    st = _get_state()
    wpair, wk2, c_scale = _prep_weights(
        theta_logsigma, Omega_mean, Omega_logsigma, Omega_eps
    )
    xq = np.clip(x * np.float32(XS), -127.0, 127.0)
    xs = np.rint(xq, out=xq).astype(np.int8)
    xg = xs.reshape(B * IN_C, H, W)

    n_runs = MC // MCP
    run_arrs = []
    for r in range(n_runs):
        cols = slice(r * WCOL, (r + 1) * WCOL)
        globals_by_name = {
            "xb": xg,
            # global [96,WCOL]/[48,WCOL]: shard k IS rows [12k:12k+12]/
            # [6k:6k+6], reassembled on device by the weight AllGather
            "wp": np.ascontiguousarray(wpair[:, cols]),
            "w2": np.ascontiguousarray(wk2[:, cols]),
        }
        concat_in = [globals_by_name[n] for n in st["in_names"]]
        # async dispatch: run r+1's upload+exec overlaps run r's fetch
        run_arrs.append(st["sharded"](*concat_in, *st["dummy_outs"]))

    final = np.empty((B, MC * 2 * OUT_C, HO, HO), np.float32)
    fin5 = final.reshape(B, n_runs, MCP * 2 * OUT_C, HO, HO)
    bpo = N_CORES // N_OUTS
    dq = np.float32(c_scale / QS)

    def _fetch(rk):
        r, k = rk
        # out{k}'s shard on core k holds the gathered batches [k*bpo,(k+1)*bpo)
        arr = dict(zip(st["out_names"], run_arrs[r]))[f"out{k}"]
        shard = None
        for s in arr.addressable_shards:
            if (s.index[0].start or 0) == k * bpo:
                shard = s.data
                break
        got = np.asarray(shard)  # [bpo, MCP, 2, OUT_C, 8, 512] int8, one RPC
        np.multiply(
            got.reshape(bpo, MCP * 2 * OUT_C, HO, HO),
            dq,
            out=fin5[k * bpo : (k + 1) * bpo, r],
        )

    from concurrent.futures import ThreadPoolExecutor

    tasks = [(r, k) for r in range(n_runs) for k in range(N_OUTS)]
    with ThreadPoolExecutor(max(2, N_OUTS)) as ex:
        list(ex.map(_fetch, tasks))
    return final


if __name__ == "__main__":
    rng = np.random.default_rng(0)
    ins = {
        "x": rng.standard_normal((B, IN_C, H, W), dtype=np.float32),
        "theta_logsigma": np.zeros((1,), np.float32),
        "Omega_mean": np.zeros((D, OUT_C), np.float32),
        "Omega_logsigma": np.full((D, OUT_C), -np.log(float(D)), np.float32),
        "Omega_eps": rng.standard_normal((MC, D, OUT_C), dtype=np.float32),
    }
    out = kernel(**ins)
    print(out.shape, out.dtype)


# revision 20
# speedup vs baseline: 1.0545x; 1.0545x over previous
"""Conv-RFF Trainium2 kernel: grouped 3x3/s2 conv (10 MC groups sharing input)
+ cos/sin random-feature epilogue, data-parallel over batch on 8 NeuronCores.

The end-to-end time is dominated by the axon tunnel (~40 MB/s, ~0.2 s/RPC),
so the kernel is organized around minimizing host<->device traffic:
  - x is uploaded in fp16 (4.2 MB), weights in fp16 (prescaled by 1/(2*pi)).
  - The conv runs as 2 accumulating fp16 matmuls per 512-pixel block into
    fp32 PSUM (row-gathered Kb layout, see below), epilogue does the
    round-to-nearest trig trick on ScalarE Sin, and writes raw cos/sin as
    fp16 (the input-dependent c_scale is applied on the host during the
    final fp16->fp32 cast, so the program never needs rebuilding).
  - An in-kernel AllGather concatenates the 8 per-core results, so the host
    fetches ONE 21 MB fp16 buffer from core 0 with a single RPC instead of
    8 shards of fp32.
  - The jitted executable, the NEFF, and the dummy output-operand buffers
    (never donated, so reusable) are cached module-globally; steady-state
    calls do no recompilation and upload only x + weights.

Conv layout per core (one batch image each):
  - Kb[96, 64, 128] fp16 in SBUF via 6 DMAs:
      partitions 0..47  = "G"  = x rows (2*oy+ki-1) unshifted
      partitions 48..95 = "Gs" = x rows (2*oy+ki-1) shifted right 1 col
  - mm1: K=96  lhsT=W[kj1|kj0] rhs=Kb[:, oy, 0:128:2]   (even cols)
    mm2: K=48  lhsT=W[kj2]     rhs=Kb[0:48, oy, 1:128:2] (odd cols)
  - Weights pre-scaled by 1/(2*pi) so PSUM holds u = ph/(2*pi) ("turns");
    round via the fp32 magic constant, then ScalarE Sin on the reduced
    argument (|arg| <= pi where HW Sin is exact), quarter-turn shift for cos.
"""

import sys

sys.path.insert(0, "/opt/trn_rl_repo")

import numpy as np

MC, IN_C, OUT_C, KK = 10, 16, 32, 3
D = IN_C * KK * KK  # 144
B, H, W = 8, 128, 128
HO = 64  # output spatial
N_RF = OUT_C * HO * HO
PI = float(np.pi)
MAGIC = float(1.5 * 2**23)
N_CORES = 8
N_OUTS = 4  # output split into 4 tensors, fetched from 4 cores in parallel
QS = 127.0  # int8 quantization scale for cos/sin values
X_INT8 = True  # quantize x to int8 for upload (scale folded into weights)
XS = 127.0 / 4.0  # int8 quantization scale for x (clips at 4 sigma)


def _build_program(JW: int = 2, SB_BUFS: int = 3, PS_BUFS: int = 3):
    import concourse.bacc as bacc
    import concourse.mybir as mybir
    from concourse.tile import TileContext

    f32 = mybir.dt.float32
    f16 = mybir.dt.float16
    i8 = mybir.dt.int8
    AF = mybir.ActivationFunctionType
    ALU = mybir.AluOpType

    nc = bacc.Bacc("TRN2", target_bir_lowering=False, num_devices=N_CORES)
    xdt = i8 if X_INT8 else f16
    xb = nc.dram_tensor("xb", [IN_C, H, W], xdt, kind="ExternalInput")
    # each core uploads 1/8th of the weights; an AllGather reassembles them
    wp = nc.dram_tensor("wp", [96 // N_CORES, 320], f16, kind="ExternalInput")
    w2 = nc.dram_tensor("w2", [48 // N_CORES, 320], f16, kind="ExternalInput")
    bpo = N_CORES // N_OUTS  # batches per output tensor
    outs = [
        nc.dram_tensor(
            f"out{k}", [bpo, MC, 2, OUT_C, 8, 512], i8, kind="ExternalOutput"
        )
        for k in range(N_OUTS)
    ]

    with TileContext(nc) as tc:
        with (
            tc.tile_pool(name="kbp", bufs=1) as kbp,
            tc.tile_pool(name="wpl", bufs=1) as wpl,
            tc.tile_pool(name="cst", bufs=1) as cst,
            tc.tile_pool(name="sb", bufs=SB_BUFS) as sb,
            tc.tile_pool(name="ob", bufs=4) as ob,
            tc.tile_pool(name="pp", bufs=PS_BUFS, space="PSUM") as pp,
            tc.tile_pool(name="dr", bufs=1, space="DRAM") as dr,
        ):
            loc = dr.tile([MC, 2, OUT_C, 8, 512], i8)
            gath = dr.tile([N_CORES, MC, 2, OUT_C, 8, 512], i8)
            kbs = kbp.tile([96, 64, 128], xdt)
            wpt = wpl.tile([96, 320], f16)
            w2t = wpl.tile([48, 320], f16)
            hpi = cst.tile([128, 1], f32)
            nc.gpsimd.memset(hpi[:], PI / 2)
            # gather the per-core weight slices (collectives can't touch I/O
            # tensors, so bounce through internal DRAM on both sides)
            wpb = dr.tile([96 // N_CORES, 320], f16)
            w2b = dr.tile([48 // N_CORES, 320], f16)
            wpg = dr.tile([96, 320], f16)
            w2g = dr.tile([48, 320], f16)
            nc.sync.dma_start(wpb[:], wp[:])
            nc.sync.dma_start(w2b[:], w2[:])
            nc.gpsimd.collective_compute(
                "AllGather",
                mybir.AluOpType.bypass,
                replica_groups=[list(range(N_CORES))],
                ins=[wpb.opt()],
                outs=[wpg.opt()],
            )
            nc.gpsimd.collective_compute(
                "AllGather",
                mybir.AluOpType.bypass,
                replica_groups=[list(range(N_CORES))],
                ins=[w2b.opt()],
                outs=[w2g.opt()],
            )
            nc.sync.dma_start(wpt[:], wpg[:])
            nc.sync.dma_start(w2t[:], w2g[:])

            # zero padding slivers: shifted col 0, and oy=0 row for ki=0 taps
            nc.gpsimd.memset(kbs[32:64, :, 0:1], 0.0)
            nc.gpsimd.memset(kbs[64:96, :, 0:1], 0.0)
            nc.gpsimd.memset(kbs[0:16, 0:1, :], 0.0)
            nc.gpsimd.memset(kbs[32:64, 0:1, :], 0.0)

            # G (unshifted) at partitions 0..47; row sets per ki
            nc.sync.dma_start(kbs[0:16, 1:64, :], xb[:, 1:127:2, :])    # ki=0
            nc.sync.dma_start(kbs[16:32, :, :], xb[:, 0:128:2, :])      # ki=1
            nc.sync.dma_start(kbs[32:48, :, :], xb[:, 1:128:2, :])      # ki=2
            # Gs (shifted right 1 col) at partitions 48..95
            nc.sync.dma_start(kbs[48:64, 1:64, 1:128], xb[:, 1:127:2, 0:127])
            nc.sync.dma_start(kbs[64:80, :, 1:128], xb[:, 0:128:2, 0:127])
            nc.sync.dma_start(kbs[80:96, :, 1:128], xb[:, 1:128:2, 0:127])
            if X_INT8:
                # upcast staged int8 image rows to fp16 for the PE array;
                # the 1/XS dequant scale is folded into the weights host-side
                kb = kbp.tile([96, 64, 128], f16)
                nc.scalar.copy(kb[:], kbs[:])
            else:
                kb = kbs

            chunks = [(0, 128), (128, 128), (256, 64)]
            for c0, mcs in chunks:
                nmc = mcs // OUT_C
                mc0 = c0 // OUT_C
                for jb in range(8 // JW):
                    ps = pp.tile([mcs, JW, 512], f32, tag="ps")
                    for jj in range(JW):
                        j = jb * JW + jj
                        nc.tensor.matmul(
                            ps[:, jj, :],
                            wpt[:, c0 : c0 + mcs],
                            kb[:, j * 8 : (j + 1) * 8, 0:128:2],
                            start=True,
                            stop=False,
                        )
                        nc.tensor.matmul(
                            ps[:, jj, :],
                            w2t[:, c0 : c0 + mcs],
                            kb[0:48, j * 8 : (j + 1) * 8, 1:128:2],
                            start=False,
                            stop=True,
                        )
                    uc = sb.tile([mcs, JW, 512], f32, tag="uc")
                    nc.scalar.activation(uc[:], ps[:], AF.Identity)
                    t1s = sb.tile([mcs, JW, 512], f32, tag="t1s")
                    nc.gpsimd.tensor_scalar(t1s[:], uc[:], MAGIC, None, op0=ALU.add)
                    t1c = sb.tile([mcs, JW, 512], f32, tag="t1c")
                    nc.gpsimd.tensor_scalar(
                        t1c[:], uc[:], 0.25, MAGIC, op0=ALU.add, op1=ALU.add
                    )
                    gs = sb.tile([mcs, JW, 512], f32, tag="gs")
                    nc.vector.scalar_tensor_tensor(
                        gs[:], t1s[:], MAGIC, uc[:], op0=ALU.subtract, op1=ALU.subtract
                    )
                    gc = sb.tile([mcs, JW, 512], f32, tag="gc")
                    nc.vector.scalar_tensor_tensor(
                        gc[:], t1c[:], MAGIC, uc[:], op0=ALU.subtract, op1=ALU.subtract
                    )
                    sn = sb.tile([mcs, JW, 512], f32, tag="sn")
                    nc.scalar.activation(sn[:], gs[:], AF.Sin, scale=-2 * PI)
                    cs = sb.tile([mcs, JW, 512], f32, tag="cs")
                    nc.scalar.activation(
                        cs[:], gc[:], AF.Sin, bias=hpi[:mcs, :], scale=-2 * PI
                    )
                    # quantize to int8: q = round(127*v) via the magic constant
                    # (value ends integer-valued in f32, so the int8 convert is
                    # exact in any rounding mode)
                    qs_t = sb.tile([mcs, JW, 512], f32, tag="qs")
                    nc.gpsimd.tensor_scalar(
                        qs_t[:], sn[:], QS, MAGIC, op0=ALU.mult, op1=ALU.add
                    )
                    qc_t = sb.tile([mcs, JW, 512], f32, tag="qc")
                    nc.gpsimd.tensor_scalar(
                        qc_t[:], cs[:], QS, MAGIC, op0=ALU.mult, op1=ALU.add
                    )
                    sn8 = ob.tile([mcs, JW, 512], i8, tag="sn8")
                    nc.vector.tensor_scalar(
                        sn8[:], qs_t[:], MAGIC, None, op0=ALU.subtract
                    )
                    cs8 = ob.tile([mcs, JW, 512], i8, tag="cs8")
                    nc.vector.tensor_scalar(
                        cs8[:], qc_t[:], MAGIC, None, op0=ALU.subtract
                    )
                    for m in range(nmc):
                        nc.sync.dma_start(
                            loc[mc0 + m, 0, :, jb * JW : (jb + 1) * JW, :],
                            cs8[m * 32 : (m + 1) * 32, :, :],
                        )
                        nc.sync.dma_start(
                            loc[mc0 + m, 1, :, jb * JW : (jb + 1) * JW, :],
                            sn8[m * 32 : (m + 1) * 32, :, :],
                        )
            nc.gpsimd.collective_compute(
                "AllGather",
                mybir.AluOpType.bypass,
                replica_groups=[list(range(N_CORES))],
                ins=[loc.opt()],
                outs=[gath.opt()],
            )
            for k in range(N_OUTS):
                nc.sync.dma_start(
                    outs[k][:], gath[k * bpo : (k + 1) * bpo]
                )
    nc.compile()
    return nc


def _prep_weights(theta_logsigma, Omega_mean, Omega_logsigma, Omega_eps):
    om = Omega_eps.astype(np.float64) * np.exp(
        Omega_logsigma.astype(np.float64) * 0.5
    ) + Omega_mean.astype(np.float64)
    wd = om.transpose(1, 0, 2).reshape(D, MC * OUT_C)  # [d, mc*32+oc]
    wscale = 2 * np.pi * (XS if X_INT8 else 1.0)
    wt = (wd / wscale).reshape(KK, KK, IN_C, MC * OUT_C)
    kj0 = wt[:, 0].reshape(48, MC * OUT_C)
    kj1 = wt[:, 1].reshape(48, MC * OUT_C)
    kj2 = wt[:, 2].reshape(48, MC * OUT_C)
    wpair = np.ascontiguousarray(
        np.concatenate([kj1, kj0], axis=0), dtype=np.float16
    )
    wk2 = np.ascontiguousarray(kj2, dtype=np.float16)
    c_scale = float(np.exp(0.5 * float(theta_logsigma[0])) / np.sqrt(N_RF))
    return wpair, wk2, c_scale


_STATE = None


def _get_state():
    global _STATE
    if _STATE is not None:
        return _STATE

    import jax
    import jax.numpy as jnp
    from jax.sharding import Mesh, NamedSharding, PartitionSpec
    from jax.experimental.shard_map import shard_map
    from concourse import mybir
    from concourse import bass2jax
    from concourse.bass2jax import _bass_exec_p, install_neuronx_cc_hook

    nc = _build_program()
    install_neuronx_cc_hook()

    partition_name = nc.partition_id_tensor.name if nc.partition_id_tensor else None
    in_names, out_names, out_avals = [], [], []
    for alloc in nc.m.functions[0].allocations:
        if not isinstance(alloc, mybir.MemoryLocationSet):
            continue
        name = alloc.memorylocations[0].name
        if alloc.kind == "ExternalInput":
            if name != partition_name:
                in_names.append(name)
        elif alloc.kind == "ExternalOutput":
            out_names.append(name)
            shape = tuple(alloc.tensor_shape)
            dtype = mybir.dt.np(alloc.dtype)
            out_avals.append(jax.core.ShapedArray(shape, dtype))
    n_params = len(in_names)
    in_names_full = in_names + out_names
    if partition_name is not None:
        in_names_full.append(partition_name)

    def _body(*args):
        operands = list(args)
        if partition_name is not None:
            operands.append(bass2jax.partition_id_tensor())
        outs = _bass_exec_p.bind(
            *operands,
            out_avals=tuple(out_avals),
            in_names=tuple(in_names_full),
            out_names=tuple(out_names),
            lowering_input_output_aliases=(),
            sim_require_finite=True,
            sim_require_nnan=True,
            nc=nc,
        )
        return tuple(outs)

    devices = jax.devices()[:N_CORES]
    mesh = Mesh(np.asarray(devices), ("core",))
    n_outs = len(out_avals)
    in_specs = (PartitionSpec("core"),) * (n_params + n_outs)
    out_specs = (PartitionSpec("core"),) * n_outs
    # No donation: output operands are dummies (the NEFF writes every
    # element of "out"), so the same device-resident zero buffers are
    # reused every call with no re-upload.
    sharded = jax.jit(
        shard_map(
            _body, mesh=mesh, in_specs=in_specs, out_specs=out_specs, check_rep=False
        ),
        keep_unused=True,
    )

    zsharding = NamedSharding(mesh, PartitionSpec("core"))
    mkzeros = jax.jit(
        lambda: tuple(
            jnp.zeros((N_CORES * a.shape[0], *a.shape[1:]), a.dtype)
            for a in out_avals
        ),
        out_shardings=(zsharding,) * n_outs,
    )
    dummy_outs = jax.block_until_ready(mkzeros())

    _STATE = {
        "sharded": sharded,
        "dummy_outs": dummy_outs,
        "in_names": in_names,
        "out_names": out_names,
    }
    return _STATE


def kernel(x, theta_logsigma, Omega_mean, Omega_logsigma, Omega_eps):
    st = _get_state()
    wpair, wk2, c_scale = _prep_weights(
        theta_logsigma, Omega_mean, Omega_logsigma, Omega_eps
    )
    if X_INT8:
        xq = np.clip(x * np.float32(XS), -127.0, 127.0)
        xs = np.rint(xq, out=xq).astype(np.int8)
    else:
        xs = np.ascontiguousarray(x, dtype=np.float16)
    globals_by_name = {
        "xb": xs.reshape(B * IN_C, H, W),
        # global [96,320]/[48,320]: shard k IS rows [12k:12k+12]/[6k:6k+6],
        # reassembled on device by the weight AllGather
        "wp": wpair,
        "w2": wk2,
    }
    concat_in = [globals_by_name[n] for n in st["in_names"]]
    out_arrs = st["sharded"](*concat_in, *st["dummy_outs"])
    final = np.empty((B, MC * 2 * OUT_C, HO, HO), np.float32)
    bpo = N_CORES // N_OUTS
    dq = np.float32(c_scale / QS)
    arr_by_name = dict(zip(st["out_names"], out_arrs))

    def _fetch(k):
        # out{k}'s shard on core k holds the gathered batches [k*bpo,(k+1)*bpo)
        arr = arr_by_name[f"out{k}"]
        shard = None
        for s in arr.addressable_shards:
            if (s.index[0].start or 0) == k * bpo:
                shard = s.data
                break
        got = np.asarray(shard)  # [bpo, MC, 2, OUT_C, 8, 512] int8, one RPC
        np.multiply(
            got.reshape(bpo, MC * 2 * OUT_C, HO, HO),
            dq,
            out=final[k * bpo : (k + 1) * bpo],
        )

    from concurrent.futures import ThreadPoolExecutor

    with ThreadPoolExecutor(N_OUTS) as ex:
        list(ex.map(_fetch, range(N_OUTS)))
    return final


if __name__ == "__main__":
    rng = np.random.default_rng(0)
    ins = {
        "x": rng.standard_normal((B, IN_C, H, W), dtype=np.float32),
        "theta_logsigma": np.zeros((1,), np.float32),
        "Omega_mean": np.zeros((D, OUT_C), np.float32),
        "Omega_logsigma": np.full((D, OUT_C), -np.log(float(D)), np.float32),
        "Omega_eps": rng.standard_normal((MC, D, OUT_C), dtype=np.float32),
    }
    out = kernel(**ins)
    print(out.shape, out.dtype)


# revision 22
# speedup vs baseline: 1.2268x; 1.1634x over previous
"""Conv-RFF Trainium2 kernel: grouped 3x3/s2 conv (10 MC groups sharing input)
+ cos/sin random-feature epilogue, data-parallel over batch on 8 NeuronCores.

The end-to-end time is dominated by the axon tunnel (~20-40 MB/s aggregate,
~0.1-0.2 s/RPC), so the kernel is organized around minimizing
host<->device traffic:
  - x is quantized to int8 (2.1 MB up; the 1/XS dequant scale is folded
    into the fp16 weights, consumed after an on-device upcast to fp16).
  - Weights upload 1/8th per core (92 KB total) and are reassembled on
    device by a small AllGather.
  - The conv runs as 2 accumulating fp16 matmuls per 512-pixel block into
    fp32 PSUM (row-gathered Kb layout, see below), the epilogue does the
    round-to-nearest trig trick on ScalarE Sin, and cos/sin are quantized
    to int8 = round(127*v) via the fp32 magic constant (exact
    round-to-nearest regardless of the convert's rounding mode).
  - An in-kernel AllGather concatenates the 8 per-core results; the output
    is split into N_OUTS tensors so the host fetches slice k from core k
    in parallel threads (10.5 MB total; parallel streams overlap the
    per-RPC latency). The input-dependent c_scale/127 dequant happens in
    one fused int8->f32 multiply on the host, so the program never needs
    rebuilding.
  - The jitted executable, the NEFF, and the dummy output-operand buffers
    (never donated, so reusable: the NEFF writes every output element) are
    cached module-globally; steady-state calls do no recompilation.

Conv layout per core (one batch image each):
  - Kb[96, 64, 128] fp16 in SBUF via 6 DMAs:
      partitions 0..47  = "G"  = x rows (2*oy+ki-1) unshifted
      partitions 48..95 = "Gs" = x rows (2*oy+ki-1) shifted right 1 col
  - mm1: K=96  lhsT=W[kj1|kj0] rhs=Kb[:, oy, 0:128:2]   (even cols)
    mm2: K=48  lhsT=W[kj2]     rhs=Kb[0:48, oy, 1:128:2] (odd cols)
  - Weights pre-scaled by 1/(2*pi) so PSUM holds u = ph/(2*pi) ("turns");
    round via the fp32 magic constant, then ScalarE Sin on the reduced
    argument (|arg| <= pi where HW Sin is exact), quarter-turn shift for cos.
"""

import os
import sys

sys.path.insert(0, "/opt/trn_rl_repo")
os.environ.setdefault("JAX_PLATFORMS", "")  # let the axon backend register

import numpy as np

MC, IN_C, OUT_C, KK = 10, 16, 32, 3
D = IN_C * KK * KK  # 144
B, H, W = 8, 128, 128
HO = 64  # output spatial
N_RF = OUT_C * HO * HO
PI = float(np.pi)
MAGIC = float(1.5 * 2**23)
N_CORES = 8
N_OUTS = 4  # output split into 4 tensors, fetched from 4 cores in parallel
QS = 127.0  # int8 quantization scale for cos/sin values
X_INT8 = True  # quantize x to int8 for upload (scale folded into weights)
XS = 127.0 / 4.0  # int8 quantization scale for x (clips at 4 sigma)


def _build_program(JW: int = 2, SB_BUFS: int = 3, PS_BUFS: int = 3):
    import concourse.bacc as bacc
    import concourse.mybir as mybir
    from concourse.tile import TileContext

    f32 = mybir.dt.float32
    f16 = mybir.dt.float16
    i8 = mybir.dt.int8
    AF = mybir.ActivationFunctionType
    ALU = mybir.AluOpType

    nc = bacc.Bacc("TRN2", target_bir_lowering=False, num_devices=N_CORES)
    xdt = i8 if X_INT8 else f16
    xb = nc.dram_tensor("xb", [IN_C, H, W], xdt, kind="ExternalInput")
    # each core uploads 1/8th of the weights; an AllGather reassembles them
    wp = nc.dram_tensor("wp", [96 // N_CORES, 320], f16, kind="ExternalInput")
    w2 = nc.dram_tensor("w2", [48 // N_CORES, 320], f16, kind="ExternalInput")
    bpo = N_CORES // N_OUTS  # batches per output tensor
    outs = [
        nc.dram_tensor(
            f"out{k}", [bpo, MC, 2, OUT_C, 8, 512], i8, kind="ExternalOutput"
        )
        for k in range(N_OUTS)
    ]

    with TileContext(nc) as tc:
        with (
            tc.tile_pool(name="kbp", bufs=1) as kbp,
            tc.tile_pool(name="wpl", bufs=1) as wpl,
            tc.tile_pool(name="cst", bufs=1) as cst,
            tc.tile_pool(name="sb", bufs=SB_BUFS) as sb,
            tc.tile_pool(name="ob", bufs=4) as ob,
            tc.tile_pool(name="pp", bufs=PS_BUFS, space="PSUM") as pp,
            tc.tile_pool(name="dr", bufs=1, space="DRAM") as dr,
        ):
            loc = dr.tile([MC, 2, OUT_C, 8, 512], i8)
            gath = dr.tile([N_CORES, MC, 2, OUT_C, 8, 512], i8)
            kbs = kbp.tile([96, 64, 128], xdt)
            wpt = wpl.tile([96, 320], f16)
            w2t = wpl.tile([48, 320], f16)
            hpi = cst.tile([128, 1], f32)
            nc.gpsimd.memset(hpi[:], PI / 2)
            # gather the per-core weight slices (collectives can't touch I/O
            # tensors, so bounce through internal DRAM on both sides)
            wpb = dr.tile([96 // N_CORES, 320], f16)
            w2b = dr.tile([48 // N_CORES, 320], f16)
            wpg = dr.tile([96, 320], f16)
            w2g = dr.tile([48, 320], f16)
            nc.sync.dma_start(wpb[:], wp[:])
            nc.sync.dma_start(w2b[:], w2[:])
            nc.gpsimd.collective_compute(
                "AllGather",
                mybir.AluOpType.bypass,
                replica_groups=[list(range(N_CORES))],
                ins=[wpb.opt()],
                outs=[wpg.opt()],
            )
            nc.gpsimd.collective_compute(
                "AllGather",
                mybir.AluOpType.bypass,
                replica_groups=[list(range(N_CORES))],
                ins=[w2b.opt()],
                outs=[w2g.opt()],
            )
            nc.sync.dma_start(wpt[:], wpg[:])
            nc.sync.dma_start(w2t[:], w2g[:])

            # zero padding slivers: shifted col 0, and oy=0 row for ki=0 taps
            nc.gpsimd.memset(kbs[32:64, :, 0:1], 0.0)
            nc.gpsimd.memset(kbs[64:96, :, 0:1], 0.0)
            nc.gpsimd.memset(kbs[0:16, 0:1, :], 0.0)
            nc.gpsimd.memset(kbs[32:64, 0:1, :], 0.0)

            # G (unshifted) at partitions 0..47; row sets per ki
            nc.sync.dma_start(kbs[0:16, 1:64, :], xb[:, 1:127:2, :])    # ki=0
            nc.sync.dma_start(kbs[16:32, :, :], xb[:, 0:128:2, :])      # ki=1
            nc.sync.dma_start(kbs[32:48, :, :], xb[:, 1:128:2, :])      # ki=2
            # Gs (shifted right 1 col) at partitions 48..95
            nc.sync.dma_start(kbs[48:64, 1:64, 1:128], xb[:, 1:127:2, 0:127])
            nc.sync.dma_start(kbs[64:80, :, 1:128], xb[:, 0:128:2, 0:127])
            nc.sync.dma_start(kbs[80:96, :, 1:128], xb[:, 1:128:2, 0:127])
            if X_INT8:
                # upcast staged int8 image rows to fp16 for the PE array;
                # the 1/XS dequant scale is folded into the weights host-side
                kb = kbp.tile([96, 64, 128], f16)
                nc.scalar.copy(kb[:], kbs[:])
            else:
                kb = kbs

            chunks = [(0, 128), (128, 128), (256, 64)]
            for c0, mcs in chunks:
                nmc = mcs // OUT_C
                mc0 = c0 // OUT_C
                for jb in range(8 // JW):
                    ps = pp.tile([mcs, JW, 512], f32, tag="ps")
                    for jj in range(JW):
                        j = jb * JW + jj
                        nc.tensor.matmul(
                            ps[:, jj, :],
                            wpt[:, c0 : c0 + mcs],
                            kb[:, j * 8 : (j + 1) * 8, 0:128:2],
                            start=True,
                            stop=False,
                        )
                        nc.tensor.matmul(
                            ps[:, jj, :],
                            w2t[:, c0 : c0 + mcs],
                            kb[0:48, j * 8 : (j + 1) * 8, 1:128:2],
                            start=False,
                            stop=True,
                        )
                    uc = sb.tile([mcs, JW, 512], f32, tag="uc")
                    nc.scalar.activation(uc[:], ps[:], AF.Identity)
                    t1s = sb.tile([mcs, JW, 512], f32, tag="t1s")
                    nc.gpsimd.tensor_scalar(t1s[:], uc[:], MAGIC, None, op0=ALU.add)
                    t1c = sb.tile([mcs, JW, 512], f32, tag="t1c")
                    nc.gpsimd.tensor_scalar(
                        t1c[:], uc[:], 0.25, MAGIC, op0=ALU.add, op1=ALU.add
                    )
                    gs = sb.tile([mcs, JW, 512], f32, tag="gs")
                    nc.vector.scalar_tensor_tensor(
                        gs[:], t1s[:], MAGIC, uc[:], op0=ALU.subtract, op1=ALU.subtract
                    )
                    gc = sb.tile([mcs, JW, 512], f32, tag="gc")
                    nc.vector.scalar_tensor_tensor(
                        gc[:], t1c[:], MAGIC, uc[:], op0=ALU.subtract, op1=ALU.subtract
                    )
                    sn = sb.tile([mcs, JW, 512], f32, tag="sn")
                    nc.scalar.activation(sn[:], gs[:], AF.Sin, scale=-2 * PI)
                    cs = sb.tile([mcs, JW, 512], f32, tag="cs")
                    nc.scalar.activation(
                        cs[:], gc[:], AF.Sin, bias=hpi[:mcs, :], scale=-2 * PI
                    )
                    # quantize to int8: q = round(127*v) via the magic constant
                    # (value ends integer-valued in f32, so the int8 convert is
                    # exact in any rounding mode)
                    qs_t = sb.tile([mcs, JW, 512], f32, tag="qs")
                    nc.gpsimd.tensor_scalar(
                        qs_t[:], sn[:], QS, MAGIC, op0=ALU.mult, op1=ALU.add
                    )
                    qc_t = sb.tile([mcs, JW, 512], f32, tag="qc")
                    nc.gpsimd.tensor_scalar(
                        qc_t[:], cs[:], QS, MAGIC, op0=ALU.mult, op1=ALU.add
                    )
                    sn8 = ob.tile([mcs, JW, 512], i8, tag="sn8")
                    nc.vector.tensor_scalar(
                        sn8[:], qs_t[:], MAGIC, None, op0=ALU.subtract
                    )
                    cs8 = ob.tile([mcs, JW, 512], i8, tag="cs8")
                    nc.vector.tensor_scalar(
                        cs8[:], qc_t[:], MAGIC, None, op0=ALU.subtract
                    )
                    for m in range(nmc):
                        nc.sync.dma_start(
                            loc[mc0 + m, 0, :, jb * JW : (jb + 1) * JW, :],
                            cs8[m * 32 : (m + 1) * 32, :, :],
                        )
                        nc.sync.dma_start(
                            loc[mc0 + m, 1, :, jb * JW : (jb + 1) * JW, :],
                            sn8[m * 32 : (m + 1) * 32, :, :],
                        )
            nc.gpsimd.collective_compute(
                "AllGather",
                mybir.AluOpType.bypass,
                replica_groups=[list(range(N_CORES))],
                ins=[loc.opt()],
                outs=[gath.opt()],
            )
            for k in range(N_OUTS):
                nc.sync.dma_start(
                    outs[k][:], gath[k * bpo : (k + 1) * bpo]
                )
    nc.compile()
    return nc


def _prep_weights(theta_logsigma, Omega_mean, Omega_logsigma, Omega_eps):
    om = Omega_eps.astype(np.float64) * np.exp(
        Omega_logsigma.astype(np.float64) * 0.5
    ) + Omega_mean.astype(np.float64)
    wd = om.transpose(1, 0, 2).reshape(D, MC * OUT_C)  # [d, mc*32+oc]
    wscale = 2 * np.pi * (XS if X_INT8 else 1.0)
    wt = (wd / wscale).reshape(KK, KK, IN_C, MC * OUT_C)
    kj0 = wt[:, 0].reshape(48, MC * OUT_C)
    kj1 = wt[:, 1].reshape(48, MC * OUT_C)
    kj2 = wt[:, 2].reshape(48, MC * OUT_C)
    wpair = np.ascontiguousarray(
        np.concatenate([kj1, kj0], axis=0), dtype=np.float16
    )
    wk2 = np.ascontiguousarray(kj2, dtype=np.float16)
    c_scale = float(np.exp(0.5 * float(theta_logsigma[0])) / np.sqrt(N_RF))
    return wpair, wk2, c_scale


_STATE = None


def _get_state():
    global _STATE
    if _STATE is not None:
        return _STATE

    import jax
    import jax.numpy as jnp
    from jax.sharding import Mesh, NamedSharding, PartitionSpec
    from jax.experimental.shard_map import shard_map
    from concourse import mybir
    from concourse import bass2jax
    from concourse.bass2jax import _bass_exec_p, install_neuronx_cc_hook

    nc = _build_program()
    install_neuronx_cc_hook()

    partition_name = nc.partition_id_tensor.name if nc.partition_id_tensor else None
    in_names, out_names, out_avals = [], [], []
    for alloc in nc.m.functions[0].allocations:
        if not isinstance(alloc, mybir.MemoryLocationSet):
            continue
        name = alloc.memorylocations[0].name
        if alloc.kind == "ExternalInput":
            if name != partition_name:
                in_names.append(name)
        elif alloc.kind == "ExternalOutput":
            out_names.append(name)
            shape = tuple(alloc.tensor_shape)
            dtype = mybir.dt.np(alloc.dtype)
            out_avals.append(jax.core.ShapedArray(shape, dtype))
    n_params = len(in_names)
    in_names_full = in_names + out_names
    if partition_name is not None:
        in_names_full.append(partition_name)

    def _body(*args):
        operands = list(args)
        if partition_name is not None:
            operands.append(bass2jax.partition_id_tensor())
        outs = _bass_exec_p.bind(
            *operands,
            out_avals=tuple(out_avals),
            in_names=tuple(in_names_full),
            out_names=tuple(out_names),
            lowering_input_output_aliases=(),
            sim_require_finite=True,
            sim_require_nnan=True,
            nc=nc,
        )
        return tuple(outs)

    devices = jax.devices()[:N_CORES]
    mesh = Mesh(np.asarray(devices), ("core",))
    n_outs = len(out_avals)
    in_specs = (PartitionSpec("core"),) * (n_params + n_outs)
    out_specs = (PartitionSpec("core"),) * n_outs
    # No donation: output operands are dummies (the NEFF writes every
    # element of "out"), so the same device-resident zero buffers are
    # reused every call with no re-upload.
    sharded = jax.jit(
        shard_map(
            _body, mesh=mesh, in_specs=in_specs, out_specs=out_specs, check_rep=False
        ),
        keep_unused=True,
    )

    zsharding = NamedSharding(mesh, PartitionSpec("core"))
    mkzeros = jax.jit(
        lambda: tuple(
            jnp.zeros((N_CORES * a.shape[0], *a.shape[1:]), a.dtype)
            for a in out_avals
        ),
        out_shardings=(zsharding,) * n_outs,
    )
    dummy_outs = jax.block_until_ready(mkzeros())

    _STATE = {
        "sharded": sharded,
        "dummy_outs": dummy_outs,
        "in_names": in_names,
        "out_names": out_names,
    }
    return _STATE


def kernel(x, theta_logsigma, Omega_mean, Omega_logsigma, Omega_eps):
    st = _get_state()
    wpair, wk2, c_scale = _prep_weights(
        theta_logsigma, Omega_mean, Omega_logsigma, Omega_eps
    )
    if X_INT8:
        xq = np.clip(x * np.float32(XS), -127.0, 127.0)
        xs = np.rint(xq, out=xq).astype(np.int8)
    else:
        xs = np.ascontiguousarray(x, dtype=np.float16)
    globals_by_name = {
        "xb": xs.reshape(B * IN_C, H, W),
        # global [96,320]/[48,320]: shard k IS rows [12k:12k+12]/[6k:6k+6],
        # reassembled on device by the weight AllGather
        "wp": wpair,
        "w2": wk2,
    }
    concat_in = [globals_by_name[n] for n in st["in_names"]]
    out_arrs = st["sharded"](*concat_in, *st["dummy_outs"])
    final = np.empty((B, MC * 2 * OUT_C, HO, HO), np.float32)
    bpo = N_CORES // N_OUTS
    dq = np.float32(c_scale / QS)
    arr_by_name = dict(zip(st["out_names"], out_arrs))

    def _fetch(k):
        # out{k}'s shard on core k holds the gathered batches [k*bpo,(k+1)*bpo)
        arr = arr_by_name[f"out{k}"]
        shard = None
        for s in arr.addressable_shards:
            if (s.index[0].start or 0) == k * bpo:
                shard = s.data
                break
        got = np.asarray(shard)  # [bpo, MC, 2, OUT_C, 8, 512] int8, one RPC
        np.multiply(
            got.reshape(bpo, MC * 2 * OUT_C, HO, HO),
            dq,
            out=final[k * bpo : (k + 1) * bpo],
        )

    from concurrent.futures import ThreadPoolExecutor

    with ThreadPoolExecutor(N_OUTS) as ex:
        list(ex.map(_fetch, range(N_OUTS)))
    return final


if __name__ == "__main__":
    rng = np.random.default_rng(0)
    ins = {
        "x": rng.standard_normal((B, IN_C, H, W), dtype=np.float32),
        "theta_logsigma": np.zeros((1,), np.float32),
        "Omega_mean": np.zeros((D, OUT_C), np.float32),
        "Omega_logsigma": np.full((D, OUT_C), -np.log(float(D)), np.float32),
        "Omega_eps": rng.standard_normal((MC, D, OUT_C), dtype=np.float32),
    }
    out = kernel(**ins)
    print(out.shape, out.dtype)


# revision 25
# speedup vs baseline: 1.2386x; 1.0096x over previous
"""Conv-RFF Trainium2 kernel: grouped 3x3/s2 conv (10 MC groups sharing input)
+ cos/sin random-feature epilogue, data-parallel over batch on 8 NeuronCores.

The end-to-end time is dominated by the axon tunnel (~20-40 MB/s aggregate,
~0.1-0.2 s/RPC), so the kernel is organized around minimizing
host<->device traffic:
  - x is quantized to int8 (2.1 MB up; the 1/XS dequant scale is folded
    into the fp16 weights, consumed after an on-device upcast to fp16).
  - Weights upload 1/8th per core (92 KB total) and are reassembled on
    device by a small AllGather.
  - The conv runs as 2 accumulating fp16 matmuls per 512-pixel block into
    fp32 PSUM (row-gathered Kb layout, see below), the epilogue does the
    round-to-nearest trig trick on ScalarE Sin, and cos/sin are quantized
    to int8 = round(127*v) via the fp32 magic constant (exact
    round-to-nearest regardless of the convert's rounding mode).
  - An in-kernel AllGather concatenates the 8 per-core results; the output
    is split into N_OUTS tensors so the host fetches slice k from core k
    in parallel threads (10.5 MB total; parallel streams overlap the
    per-RPC latency). The input-dependent c_scale/127 dequant happens in
    one fused int8->f32 multiply on the host, so the program never needs
    rebuilding.
  - The jitted executable, the NEFF, and the dummy output-operand buffers
    (never donated, so reusable: the NEFF writes every output element) are
    cached module-globally; steady-state calls do no recompilation.

Conv layout per core (one batch image each):
  - Kb[96, 64, 128] fp16 in SBUF via 6 DMAs:
      partitions 0..47  = "G"  = x rows (2*oy+ki-1) unshifted
      partitions 48..95 = "Gs" = x rows (2*oy+ki-1) shifted right 1 col
  - mm1: K=96  lhsT=W[kj1|kj0] rhs=Kb[:, oy, 0:128:2]   (even cols)
    mm2: K=48  lhsT=W[kj2]     rhs=Kb[0:48, oy, 1:128:2] (odd cols)
  - Weights pre-scaled by 1/(2*pi) so PSUM holds u = ph/(2*pi) ("turns");
    round via the fp32 magic constant, then ScalarE Sin on the reduced
    argument (|arg| <= pi where HW Sin is exact), quarter-turn shift for cos.
"""

import os
import sys

sys.path.insert(0, "/opt/trn_rl_repo")
os.environ.setdefault("JAX_PLATFORMS", "")  # let the axon backend register

import numpy as np

MC, IN_C, OUT_C, KK = 10, 16, 32, 3
D = IN_C * KK * KK  # 144
B, H, W = 8, 128, 128
HO = 64  # output spatial
N_RF = OUT_C * HO * HO
PI = float(np.pi)
MAGIC = float(1.5 * 2**23)
N_CORES = 8
N_OUTS = 4  # output split into 4 tensors, fetched from 4 cores in parallel
QS = 127.0  # int8 quantization scale for cos/sin values
X_INT8 = True  # quantize x to int8 for upload (scale folded into weights)
XS = 127.0 / 4.0  # int8 quantization scale for x (clips at 4 sigma)


def _build_program(JW: int = 2, SB_BUFS: int = 3, PS_BUFS: int = 3):
    import concourse.bacc as bacc
    import concourse.mybir as mybir
    from concourse.tile import TileContext

    f32 = mybir.dt.float32
    f16 = mybir.dt.float16
    i8 = mybir.dt.int8
    AF = mybir.ActivationFunctionType
    ALU = mybir.AluOpType

    nc = bacc.Bacc("TRN2", target_bir_lowering=False, num_devices=N_CORES)
    xdt = i8 if X_INT8 else f16
    xb = nc.dram_tensor("xb", [IN_C, H, W], xdt, kind="ExternalInput")
    # each core uploads 1/8th of the weights; an AllGather reassembles them
    wp = nc.dram_tensor("wp", [96 // N_CORES, 320], f16, kind="ExternalInput")
    w2 = nc.dram_tensor("w2", [48 // N_CORES, 320], f16, kind="ExternalInput")
    bpo = N_CORES // N_OUTS  # batches per output tensor
    outs = [
        nc.dram_tensor(
            f"out{k}", [bpo, MC, 2, OUT_C, 8, 512], i8, kind="ExternalOutput"
        )
        for k in range(N_OUTS)
    ]

    with TileContext(nc) as tc:
        with (
            tc.tile_pool(name="kbp", bufs=1) as kbp,
            tc.tile_pool(name="wpl", bufs=1) as wpl,
            tc.tile_pool(name="cst", bufs=1) as cst,
            tc.tile_pool(name="sb", bufs=SB_BUFS) as sb,
            tc.tile_pool(name="ob", bufs=4) as ob,
            tc.tile_pool(name="pp", bufs=PS_BUFS, space="PSUM") as pp,
            tc.tile_pool(name="dr", bufs=1, space="DRAM") as dr,
        ):
            loc = dr.tile([MC, 2, OUT_C, 8, 512], i8)
            gath = dr.tile([N_CORES, MC, 2, OUT_C, 8, 512], i8)
            kbs = kbp.tile([96, 64, 128], xdt)
            wpt = wpl.tile([96, 320], f16)
            w2t = wpl.tile([48, 320], f16)
            hpi = cst.tile([128, 1], f32)
            nc.gpsimd.memset(hpi[:], PI / 2)
            # gather the per-core weight slices (collectives can't touch I/O
            # tensors, so bounce through internal DRAM on both sides)
            wpb = dr.tile([96 // N_CORES, 320], f16)
            w2b = dr.tile([48 // N_CORES, 320], f16)
            wpg = dr.tile([96, 320], f16)
            w2g = dr.tile([48, 320], f16)
            nc.sync.dma_start(wpb[:], wp[:])
            nc.sync.dma_start(w2b[:], w2[:])
            nc.gpsimd.collective_compute(
                "AllGather",
                mybir.AluOpType.bypass,
                replica_groups=[list(range(N_CORES))],
                ins=[wpb.opt()],
                outs=[wpg.opt()],
            )
            nc.gpsimd.collective_compute(
                "AllGather",
                mybir.AluOpType.bypass,
                replica_groups=[list(range(N_CORES))],
                ins=[w2b.opt()],
                outs=[w2g.opt()],
            )
            nc.sync.dma_start(wpt[:], wpg[:])
            nc.sync.dma_start(w2t[:], w2g[:])

            # zero padding slivers: shifted col 0, and oy=0 row for ki=0 taps
            nc.gpsimd.memset(kbs[32:64, :, 0:1], 0.0)
            nc.gpsimd.memset(kbs[64:96, :, 0:1], 0.0)
            nc.gpsimd.memset(kbs[0:16, 0:1, :], 0.0)
            nc.gpsimd.memset(kbs[32:64, 0:1, :], 0.0)

            # G (unshifted) at partitions 0..47; row sets per ki
            nc.sync.dma_start(kbs[0:16, 1:64, :], xb[:, 1:127:2, :])    # ki=0
            nc.sync.dma_start(kbs[16:32, :, :], xb[:, 0:128:2, :])      # ki=1
            nc.sync.dma_start(kbs[32:48, :, :], xb[:, 1:128:2, :])      # ki=2
            # Gs (shifted right 1 col) at partitions 48..95
            nc.sync.dma_start(kbs[48:64, 1:64, 1:128], xb[:, 1:127:2, 0:127])
            nc.sync.dma_start(kbs[64:80, :, 1:128], xb[:, 0:128:2, 0:127])
            nc.sync.dma_start(kbs[80:96, :, 1:128], xb[:, 1:128:2, 0:127])
            if X_INT8:
                # upcast staged int8 image rows to fp16 for the PE array;
                # the 1/XS dequant scale is folded into the weights host-side
                kb = kbp.tile([96, 64, 128], f16)
                nc.scalar.copy(kb[:], kbs[:])
            else:
                kb = kbs

            chunks = [(0, 128), (128, 128), (256, 64)]
            for c0, mcs in chunks:
                nmc = mcs // OUT_C
                mc0 = c0 // OUT_C
                for jb in range(8 // JW):
                    ps = pp.tile([mcs, JW, 512], f32, tag="ps")
                    for jj in range(JW):
                        j = jb * JW + jj
                        nc.tensor.matmul(
                            ps[:, jj, :],
                            wpt[:, c0 : c0 + mcs],
                            kb[:, j * 8 : (j + 1) * 8, 0:128:2],
                            start=True,
                            stop=False,
                        )
                        nc.tensor.matmul(
                            ps[:, jj, :],
                            w2t[:, c0 : c0 + mcs],
                            kb[0:48, j * 8 : (j + 1) * 8, 1:128:2],
                            start=False,
                            stop=True,
                        )
                    uc = sb.tile([mcs, JW, 512], f32, tag="uc")
                    nc.scalar.activation(uc[:], ps[:], AF.Identity)
                    t1s = sb.tile([mcs, JW, 512], f32, tag="t1s")
                    nc.gpsimd.tensor_scalar(t1s[:], uc[:], MAGIC, None, op0=ALU.add)
                    t1c = sb.tile([mcs, JW, 512], f32, tag="t1c")
                    nc.gpsimd.tensor_scalar(
                        t1c[:], uc[:], 0.25, MAGIC, op0=ALU.add, op1=ALU.add
                    )
                    gs = sb.tile([mcs, JW, 512], f32, tag="gs")
                    nc.vector.scalar_tensor_tensor(
                        gs[:], t1s[:], MAGIC, uc[:], op0=ALU.subtract, op1=ALU.subtract
                    )
                    gc = sb.tile([mcs, JW, 512], f32, tag="gc")
                    nc.vector.scalar_tensor_tensor(
                        gc[:], t1c[:], MAGIC, uc[:], op0=ALU.subtract, op1=ALU.subtract
                    )
                    sn = sb.tile([mcs, JW, 512], f32, tag="sn")
                    nc.scalar.activation(sn[:], gs[:], AF.Sin, scale=-2 * PI)
                    cs = sb.tile([mcs, JW, 512], f32, tag="cs")
                    nc.scalar.activation(
                        cs[:], gc[:], AF.Sin, bias=hpi[:mcs, :], scale=-2 * PI
                    )
                    # quantize to int8: q = round(127*v) via the magic constant
                    # (value ends integer-valued in f32, so the int8 convert is
                    # exact in any rounding mode)
                    qs_t = sb.tile([mcs, JW, 512], f32, tag="qs")
                    nc.gpsimd.tensor_scalar(
                        qs_t[:], sn[:], QS, MAGIC, op0=ALU.mult, op1=ALU.add
                    )
                    qc_t = sb.tile([mcs, JW, 512], f32, tag="qc")
                    nc.gpsimd.tensor_scalar(
                        qc_t[:], cs[:], QS, MAGIC, op0=ALU.mult, op1=ALU.add
                    )
                    sn8 = ob.tile([mcs, JW, 512], i8, tag="sn8")
                    nc.vector.tensor_scalar(
                        sn8[:], qs_t[:], MAGIC, None, op0=ALU.subtract
                    )
                    cs8 = ob.tile([mcs, JW, 512], i8, tag="cs8")
                    nc.vector.tensor_scalar(
                        cs8[:], qc_t[:], MAGIC, None, op0=ALU.subtract
                    )
                    for m in range(nmc):
                        nc.sync.dma_start(
                            loc[mc0 + m, 0, :, jb * JW : (jb + 1) * JW, :],
                            cs8[m * 32 : (m + 1) * 32, :, :],
                        )
                        nc.sync.dma_start(
                            loc[mc0 + m, 1, :, jb * JW : (jb + 1) * JW, :],
                            sn8[m * 32 : (m + 1) * 32, :, :],
                        )
            nc.gpsimd.collective_compute(
                "AllGather",
                mybir.AluOpType.bypass,
                replica_groups=[list(range(N_CORES))],
                ins=[loc.opt()],
                outs=[gath.opt()],
            )
            for k in range(N_OUTS):
                nc.sync.dma_start(
                    outs[k][:], gath[k * bpo : (k + 1) * bpo]
                )
    nc.compile()
    return nc


def _prep_weights(theta_logsigma, Omega_mean, Omega_logsigma, Omega_eps):
    om = Omega_eps.astype(np.float64) * np.exp(
        Omega_logsigma.astype(np.float64) * 0.5
    ) + Omega_mean.astype(np.float64)
    wd = om.transpose(1, 0, 2).reshape(D, MC * OUT_C)  # [d, mc*32+oc]
    wscale = 2 * np.pi * (XS if X_INT8 else 1.0)
    wt = (wd / wscale).reshape(KK, KK, IN_C, MC * OUT_C)
    kj0 = wt[:, 0].reshape(48, MC * OUT_C)
    kj1 = wt[:, 1].reshape(48, MC * OUT_C)
    kj2 = wt[:, 2].reshape(48, MC * OUT_C)
    wpair = np.ascontiguousarray(
        np.concatenate([kj1, kj0], axis=0), dtype=np.float16
    )
    wk2 = np.ascontiguousarray(kj2, dtype=np.float16)
    c_scale = float(np.exp(0.5 * float(theta_logsigma[0])) / np.sqrt(N_RF))
    return wpair, wk2, c_scale


_STATE = None


def _get_state():
    global _STATE
    if _STATE is not None:
        return _STATE

    import jax
    import jax.numpy as jnp
    from jax.sharding import Mesh, NamedSharding, PartitionSpec
    from jax.experimental.shard_map import shard_map
    from concourse import mybir
    from concourse import bass2jax
    from concourse.bass2jax import _bass_exec_p, install_neuronx_cc_hook

    nc = _build_program()
    install_neuronx_cc_hook()

    partition_name = nc.partition_id_tensor.name if nc.partition_id_tensor else None
    in_names, out_names, out_avals = [], [], []
    for alloc in nc.m.functions[0].allocations:
        if not isinstance(alloc, mybir.MemoryLocationSet):
            continue
        name = alloc.memorylocations[0].name
        if alloc.kind == "ExternalInput":
            if name != partition_name:
                in_names.append(name)
        elif alloc.kind == "ExternalOutput":
            out_names.append(name)
            shape = tuple(alloc.tensor_shape)
            dtype = mybir.dt.np(alloc.dtype)
            out_avals.append(jax.core.ShapedArray(shape, dtype))
    n_params = len(in_names)
    in_names_full = in_names + out_names
    if partition_name is not None:
        in_names_full.append(partition_name)

    def _body(*args):
        operands = list(args)
        if partition_name is not None:
            operands.append(bass2jax.partition_id_tensor())
        outs = _bass_exec_p.bind(
            *operands,
            out_avals=tuple(out_avals),
            in_names=tuple(in_names_full),
            out_names=tuple(out_names),
            lowering_input_output_aliases=(),
            sim_require_finite=True,
            sim_require_nnan=True,
            nc=nc,
        )
        return tuple(outs)

    devices = jax.devices()[:N_CORES]
    mesh = Mesh(np.asarray(devices), ("core",))
    n_outs = len(out_avals)
    in_specs = (PartitionSpec("core"),) * (n_params + n_outs)
    out_specs = (PartitionSpec("core"),) * n_outs
    # No donation: output operands are dummies (the NEFF writes every
    # element of "out"), so the same device-resident zero buffers are
    # reused every call with no re-upload.
    sharded = jax.jit(
        shard_map(
            _body, mesh=mesh, in_specs=in_specs, out_specs=out_specs, check_rep=False
        ),
        keep_unused=True,
    )

    zsharding = NamedSharding(mesh, PartitionSpec("core"))
    mkzeros = jax.jit(
        lambda: tuple(
            jnp.zeros((N_CORES * a.shape[0], *a.shape[1:]), a.dtype)
            for a in out_avals
        ),
        out_shardings=(zsharding,) * n_outs,
    )
    dummy_outs = jax.block_until_ready(mkzeros())

    from concurrent.futures import ThreadPoolExecutor

    _STATE = {
        "sharded": sharded,
        "dummy_outs": dummy_outs,
        "in_names": in_names,
        "out_names": out_names,
        "pool": ThreadPoolExecutor(N_OUTS),
        "xq_scratch": np.empty((B, IN_C, H, W), np.float32),
        "xs_scratch": np.empty((B, IN_C, H, W), np.int8),
    }
    return _STATE


def kernel(x, theta_logsigma, Omega_mean, Omega_logsigma, Omega_eps):
    st = _get_state()
    wpair, wk2, c_scale = _prep_weights(
        theta_logsigma, Omega_mean, Omega_logsigma, Omega_eps
    )
    if X_INT8:
        xq = st["xq_scratch"]
        np.multiply(x, np.float32(XS), out=xq)
        np.clip(xq, -127.0, 127.0, out=xq)
        np.rint(xq, out=xq)
        xs = st["xs_scratch"]
        np.copyto(xs, xq, casting="unsafe")
    else:
        xs = np.ascontiguousarray(x, dtype=np.float16)
    globals_by_name = {
        "xb": xs.reshape(B * IN_C, H, W),
        # global [96,320]/[48,320]: shard k IS rows [12k:12k+12]/[6k:6k+6],
        # reassembled on device by the weight AllGather
        "wp": wpair,
        "w2": wk2,
    }
    concat_in = [globals_by_name[n] for n in st["in_names"]]
    out_arrs = st["sharded"](*concat_in, *st["dummy_outs"])
    final = np.empty((B, MC * 2 * OUT_C, HO, HO), np.float32)
    bpo = N_CORES // N_OUTS
    dq = np.float32(c_scale / QS)
    arr_by_name = dict(zip(st["out_names"], out_arrs))

    def _fetch(k):
        # out{k}'s shard on core k holds the gathered batches [k*bpo,(k+1)*bpo)
        arr = arr_by_name[f"out{k}"]
        shard = None
        for s in arr.addressable_shards:
            if (s.index[0].start or 0) == k * bpo:
                shard = s.data
                break
        got = np.asarray(shard)  # [bpo, MC, 2, OUT_C, 8, 512] int8, one RPC
        np.multiply(
            got.reshape(bpo, MC * 2 * OUT_C, HO, HO),
            dq,
            out=final[k * bpo : (k + 1) * bpo],
        )

    list(st["pool"].map(_fetch, range(N_OUTS)))
    return final


if __name__ == "__main__":
    rng = np.random.default_rng(0)
    ins = {
        "x": rng.standard_normal((B, IN_C, H, W), dtype=np.float32),
        "theta_logsigma": np.zeros((1,), np.float32),
        "Omega_mean": np.zeros((D, OUT_C), np.float32),
        "Omega_logsigma": np.full((D, OUT_C), -np.log(float(D)), np.float32),
        "Omega_eps": rng.standard_normal((MC, D, OUT_C), dtype=np.float32),
    }
    out = kernel(**ins)
    print(out.shape, out.dtype)
